# revision 33
# baseline (speedup 1.0000x reference)
"""MoE clustered attention kernel for Trainium2 (8 NeuronCores).

Problem: B=2, LQ=LK=2048, D=1024, H=16 heads (DH=64), M=8 clusters.
Each query/key token is routed (argmax of X @ Wr) to one of 8 clusters;
attention is only computed within a cluster (block-sparse attention).

Strategy (v4)
-------------
Host side:
  * compute router assignments with numpy fp32,
  * gather tokens by cluster into contiguous ranges (queries rounded to
    4, keys to multiples of 128) so one SPMD program serves both
    batches; everything is fp16,
  * pre-transpose X to [D, L] block-major; pre-pack weights into the
    on-chip [128, 2048] layout so weight DMAs are fully contiguous,
  * a per-key-chunk "real key" indicator column rides in vA at slot 64:
    padded keys contribute exp(0)*0 to the denominator and 0*v to the
    numerator, so no mask rows are needed at all (K=64 contraction).

Device side (per core; core = batch * 4 + head_group, 4 heads each):
  * weight DMAs issue on the Scalar hardware-DGE queue, X blocks on the
    Sync queue, so the first projection matmul starts ~10us earlier,
  * per-PAIR qT/kT projections in transposed layout [128, L] (head 2p
    on partitions 0:64, head 2p+1 on 64:128) with N<=512 moving
    matmuls; ONE full-partition evacuation per pair-block on VectorE;
    v in natural layout with the indicator column making the ctx
    matmul emit the softmax denominator as row 64,
  * attention iterates cluster-outer / pair-inner: scores for both
    heads of the pair, batched exp on ScalarE (ScalarE runs ONLY exp
    during attention - no FIFO head-of-line blocking), ctx matmul,
    then the denominator row is staged to partition 0 (VectorE),
    reciprocal'd (VectorE), broadcast (GpSimdE) and multiplied into
    the ctx evacuation in a single fused VectorE op,
  * output projection is transposed (stationary = Wo 128-dim chunk,
    moving = the cluster's nqp tokens): 16 matmuls of N=nqp per
    cluster, evacuated into a [128, 8*512] staging tile and DMA'd out
    in one descriptor per cluster; chunks of the previous cluster are
    interleaved into the exp-latency slots of the current cluster.
Host sums the 4 head-group partials per batch in fp32 and un-permutes.
"""

import numpy as np

import concourse.bacc as bacc
import concourse.tile as tile
import concourse.mybir as mybir
import concourse.hw_specs as hw_specs
from concourse.bass_utils import run_bass_kernel_spmd

F32 = mybir.dt.float32
F16 = mybir.dt.float16
EXP = mybir.ActivationFunctionType.Exp
LN = mybir.ActivationFunctionType.Ln
MULT = mybir.AluOpType.mult

H = 16            # total heads
HPC = 4           # heads per core
N_CORES = 8

MMDT = F16
NPDT = np.float16

# Route Exp and Ln to the one ACT table set that contains both, so the
# table-load insertion pass emits a single load instead of thrashing
# between exp_and_others and natural_log on every call.
_orig_get_activation_tables = hw_specs.get_activation_tables


def _patched_get_activation_tables(arch):
    out = {}
    for name, fns in _orig_get_activation_tables(arch).items():
        fns = set(fns)
        if name != "natural_log_exp_and_others":
            fns.discard(EXP)
            fns.discard(LN)
        out[name] = fns
    return out


bacc.get_activation_tables = _patched_get_activation_tables


def _ceil_to(x, m):
    return (x + m - 1) // m * m


def pack_x(x):
    # [L, D] -> block-major [128, NB, 8, 512] so one 512-token block is
    # an 8KB-contiguous slab per partition
    L = x.shape[0]
    nb = _ceil_to(L, 512) // 512
    xp = np.zeros((128, 8, nb * 512), np.float32)
    xp[:, :, :L] = x.T.reshape(8, 128, L).transpose(1, 0, 2)
    return np.ascontiguousarray(
        xp.reshape(128, 8, nb, 512).transpose(0, 2, 1, 3).reshape(128, -1)
    ).astype(NPDT)


def pack_w(w):
    # [1024, 256] -> on-chip [128, 8*256] with w[p, d*256+m] = W[d*128+p, m]
    return np.ascontiguousarray(
        w.reshape(8, 128, 256).transpose(1, 0, 2).reshape(128, 2048)).astype(NPDT)


def pack_wo(w):
    # [256, 1024] -> [128, 2*1024] with wo[p, n*1024+m] = W[n*128+p, m]
    return np.ascontiguousarray(
        w.reshape(2, 128, 1024).transpose(1, 0, 2).reshape(128, 2048)).astype(NPDT)


def _plan(aq, ak, M):
    """Common (cross-batch) padded cluster geometry."""
    B = aq.shape[0]
    nq = np.array([[int((aq[b] == c).sum()) for c in range(M)] for b in range(B)])
    nk = np.array([[int((ak[b] == c).sum()) for c in range(M)] for b in range(B)])
    NQP = [max(64, _ceil_to(int(nq[:, c].max()), 4)) for c in range(M)]
    NKP = [_ceil_to(max(128, int(nk[:, c].max())), 128) for c in range(M)]
    qoff = np.concatenate([[0], np.cumsum(NQP)])
    koff = np.concatenate([[0], np.cumsum(NKP)])
    LQG = _ceil_to(int(qoff[-1]), 4)
    NKG = _ceil_to(int(koff[-1]), 128)
    return NQP, NKP, qoff[:-1].tolist(), koff[:-1].tolist(), LQG, NKG


def _build_program(NQP, NKP, qoffs, koffs, LQG, NKG, D, kreal_max):
    nc = bacc.Bacc("TRN2", target_bir_lowering=False, debug=False)
    ND = D // 128
    NBQ = _ceil_to(LQG, 512) // 512
    NBK = _ceil_to(NKG, 512) // 512
    # block-major layout [128, nblocks, 8, 512]: one 512-token block is an
    # 8KB-contiguous slab per partition -> 128 DMA descriptors instead of 1024
    XQT = nc.dram_tensor("XQT", [128, NBQ * ND * 512], MMDT, kind="ExternalInput").ap()
    XKT = nc.dram_tensor("XKT", [128, NBK * ND * 512], MMDT, kind="ExternalInput").ap()
    XVT = nc.dram_tensor("XVT", [128, NBK * ND * 512], MMDT, kind="ExternalInput").ap()
    WQ = nc.dram_tensor("WQ", [128, 2048], MMDT, kind="ExternalInput").ap()
    WK = nc.dram_tensor("WK", [128, 2048], MMDT, kind="ExternalInput").ap()
    WV = nc.dram_tensor("WV", [128, 2048], MMDT, kind="ExternalInput").ap()
    WO = nc.dram_tensor("WO", [128, 2048], MMDT, kind="ExternalInput").ap()
    NVC = NKG // 128       # value token chunks
    ONE = nc.dram_tensor("ONE", [128, NVC * HPC], MMDT, kind="ExternalInput").ap()
    # transposed output: OUTT[p, dc*LQG + t] = out[t, dc*128 + p]
    OUTT = nc.dram_tensor("OUTT", [128, 8 * LQG], F16, kind="ExternalOutput").ap()

    M = len(NQP)

    def xview(xdram, nb):
        return xdram.rearrange("p (b n m) -> p b n m", b=nb, n=ND)

    with tile.TileContext(nc) as tc:
        with (
            tc.tile_pool(name="weights", bufs=1) as wpool,
            tc.tile_pool(name="proj_out", bufs=1) as projpool,
            tc.tile_pool(name="psA", bufs=2, space="PSUM") as psA,
            tc.tile_pool(name="psB", bufs=2, space="PSUM") as psB,
            tc.tile_pool(name="psC", bufs=2, space="PSUM") as psC,
        ):
            wq = wpool.tile([128, 2048], MMDT, tag="wq")
            wk = wpool.tile([128, 2048], MMDT, tag="wk")
            wv = wpool.tile([128, 2048], MMDT, tag="wv")
            wo = wpool.tile([128, 2048], MMDT, tag="wo")

            # pair-layout projections: head 2p on partitions 0:64,
            # head 2p+1 on partitions 64:128
            qT = [projpool.tile([128, LQG], MMDT, tag=f"qT{p}", name=f"qT{p}")
                  for p in range(2)]
            kT = [projpool.tile([128, NKG], MMDT, tag=f"kT{p}", name=f"kT{p}")
                  for p in range(2)]
            vA = projpool.tile([128, NVC * 260], MMDT, tag="vA")

            # wq/wk/ones ride the Scalar hwdge queue so the Sync queue's
            # first issues are the first X sub-blocks.  Later weights are
            # issued from the Sync queue BETWEEN x-blocks: the DMA engines
            # round-robin everything enqueued, so issue order is the only
            # control over which transfers get bandwidth first.
            nc.scalar.dma_start(wq[:, 0:1024], WQ[:, 0:1024])
            nc.scalar.dma_start(wq[:, 1024:2048], WQ[:, 1024:2048])
            nc.scalar.dma_start(wk[:], WK)

            vA_c = vA[:].rearrange("p (c h e) -> p c h e", c=NVC, h=HPC)
            # real-key indicator column (vA slot 64 per chunk/head): DMA
            # contiguously into a staging tile (a strided 2-byte-element DMA
            # costs ~33us in descriptor processing!), then one fast DVE
            # strided copy into vA.
            ones_sb = projpool.tile([128, NVC * HPC], MMDT, tag="ones_sb")
            nc.scalar.dma_start(ones_sb[:], ONE)
            nc.vector.tensor_copy(vA_c[:, :, :, 64:65], ones_sb[:])

            # ---- projections ----
            # Block DMA and block compute are DECOUPLED: pump_dma() issues a
            # block's DMA (+ queues its compute thunks) as early as possible;
            # thunks are emitted either in bulk (drain, before the cluster
            # that needs them) or one at a time inside the attention's
            # exp-latency slots, so TensorE never idles long enough for the
            # HAM clock gate to re-throttle.
            with tc.tile_pool(name="xin", bufs=8) as xpool:
                def q_pair(xt, off, pair):
                    w = min(512, LQG - off)
                    ps = psA.tile([128, 512], F32, tag="psproj")
                    for d in range(ND):
                        nc.tensor.matmul(
                            ps[:, :w],
                            wq[:, d * 256 + pair * 128: d * 256 + (pair + 1) * 128],
                            xt[:, d, :w],
                            start=(d == 0), stop=(d == ND - 1))
                    nc.vector.tensor_copy(qT[pair][:, off:off + w], ps[:, :w])

                def k_pair(xt, off, pair):
                    w = min(512, NKG - off)
                    ps = psA.tile([128, 512], F32, tag="psproj")
                    for d in range(ND):
                        nc.tensor.matmul(
                            ps[:, :w],
                            wk[:, d * 256 + pair * 128: d * 256 + (pair + 1) * 128],
                            xt[:, d, :w],
                            start=(d == 0), stop=(d == ND - 1))
                    nc.vector.tensor_copy(kT[pair][:, off:off + w], ps[:, :w])

                def v_sub(xt, off, sub2):
                    w = min(512, NKG - off)
                    n2 = min(2, w // 128 - sub2)
                    tc128 = off // 128 + sub2
                    ps = psA.tile([128, 512], F32, tag="psproj")
                    for s in range(n2):
                        for d in range(ND):
                            nc.tensor.matmul(ps[:, s * 256:(s + 1) * 256],
                                             xt[:, d, (sub2 + s) * 128:(sub2 + s + 1) * 128],
                                             wv[:, d * 256:(d + 1) * 256],
                                             start=(d == 0), stop=(d == ND - 1))
                    nc.vector.tensor_copy(
                        vA_c[:, tc128:tc128 + n2, :, 0:64],
                        ps[:, :n2 * 256].rearrange("p (c h e) -> p c h e", c=n2, h=HPC))

                # static padding segments of the gathered key axis: tokens
                # [koffs[c]+kreal_max[c], koffs[c]+NKP[c]) are zero for BOTH
                # batches - skip their DMA entirely and zero them on-chip.
                pad_ranges = []
                for c in range(M):
                    a = koffs[c] + kreal_max[c]
                    bb = koffs[c] + NKP[c]
                    if a < bb:
                        pad_ranges.append((a, bb))

                def kv_dma(xt, xdram, off, w):
                    xv = xview(xdram, NBK)
                    pos = off
                    for (a, bb) in pad_ranges:
                        a2, b2 = max(a, off), min(bb, off + w)
                        if a2 >= b2:
                            continue
                        if pos < a2:
                            nc.sync.dma_start(
                                xt[:, :, pos - off:a2 - off],
                                xv[:, off // 512, :, pos - off:a2 - off])
                        nc.gpsimd.memset(xt[:, :, a2 - off:b2 - off], 0.0)
                        pos = b2
                    if pos < off + w:
                        nc.sync.dma_start(
                            xt[:, :, pos - off:w],
                            xv[:, off // 512, :, pos - off:w])

                # interleaved block DMA schedule: q/k/v alternate so the DMA
                # engines always work ~1 block ahead of the PE; weights are
                # enqueued between blocks (the DMA engines round-robin all
                # enqueued transfers, so issue order controls bandwidth).
                sched = []
                nqb = (LQG + 511) // 512
                nkb = (NKG + 511) // 512
                for i in range(max(nqb, nkb)):
                    if i < nqb:
                        sched.append(("q", i * 512))
                    if i < nkb:
                        if i == 0:
                            sched.append(("wk", 0))
                        sched.append(("k", i * 512))
                        if i == 0:
                            sched.append(("wv", 0))
                        if i == 1:
                            sched.append(("wo", 0))
                        sched.append(("v", i * 512))
                dma_done = {"q": 0, "k": 0, "v": 0}
                comp_done = {"q": 0, "k": 0, "v": 0}
                pending = {"q": [], "k": [], "v": []}
                sched_pos = [0]

                def issue_next():
                    kind, off = sched[sched_pos[0]]
                    sched_pos[0] += 1
                    if kind == "wk":
                        nc.sync.dma_start(wk[:], WK)
                        return
                    if kind == "wv":
                        nc.sync.dma_start(wv[:], WV)
                        return
                    if kind == "wo":
                        nc.sync.dma_start(wo[:], WO)
                        return
                    xt = xpool.tile([128, ND, 512], MMDT, tag="xt", name="xt")
                    if kind == "q":
                        xv = xview(XQT, NBQ)
                        if off == 0:
                            # split the first block so the first matmul only
                            # waits for a quarter of it
                            for sd in range(0, ND, 2):
                                nc.sync.dma_start(xt[:, sd:sd + 2, :],
                                                  xv[:, 0, sd:sd + 2, :])
                        else:
                            nc.sync.dma_start(xt[:, :, :], xv[:, off // 512, :, :])
                        thunks = [lambda p=p, x=xt, o=off: q_pair(x, o, p)
                                  for p in range(2)]
                    elif kind == "k":
                        kv_dma(xt, XKT, off, min(512, NKG - off))
                        thunks = [lambda p=p, x=xt, o=off: k_pair(x, o, p)
                                  for p in range(2)]
                    else:
                        w = min(512, NKG - off)
                        kv_dma(xt, XVT, off, w)
                        thunks = [lambda s=s, x=xt, o=off: v_sub(x, o, s)
                                  for s in range(0, w // 128, 2)]
                    pending[kind].append((off, thunks))
                    dma_done[kind] = off + 512

                def pump_dma(qneed, kvneed):
                    while (dma_done["q"] < qneed or dma_done["k"] < kvneed
                           or dma_done["v"] < kvneed) and sched_pos[0] < len(sched):
                        issue_next()

                def drain_one(kind):
                    off, thunks = pending[kind][0]
                    thunks.pop(0)()
                    if not thunks:
                        pending[kind].pop(0)
                        comp_done[kind] = off + 512

                def drain_compute(qneed, kvneed):
                    for kind, need in (("q", qneed), ("k", kvneed), ("v", kvneed)):
                        while comp_done[kind] < need and pending[kind]:
                            drain_one(kind)

                def pull_any():
                    for kind in ("k", "v", "q"):
                        if pending[kind]:
                            drain_one(kind)
                            return

                # ---- cluster-ordered K/V projection + attention + output ----
                # K/V blocks are emitted just before the first cluster that
                # needs them, so TensorE runs one dense stream through the
                # whole kernel (keeps the HAM clock warm); ctx lives in small
                # per-cluster tiles so cluster c's output projection never
                # write-after-read blocks cluster c+1's ctx.
                with tc.tile_pool(name="epool", bufs=6) as epool, \
                     tc.tile_pool(name="ccpool", bufs=3) as ccpool, \
                     tc.tile_pool(name="btpool", bufs=3) as btpool, \
                     tc.tile_pool(name="bbpool", bufs=4) as bbpool, \
                     tc.tile_pool(name="outsb", bufs=2) as opool:
                    kmax = koffs[M - 1] + NKP[M - 1]
                    state = {}

                    def attn(c, pull):
                        """Scores/exp/ctx/normalize for the 4 heads of
                        cluster c, pair at a time.  `pull()` emits one unit
                        of ready TensorE work (prev cluster's oproj or a
                        pending projection thunk) into each exp/denominator
                        latency slot."""
                        qo, nqp = qoffs[c], NQP[c]
                        nkc = NKP[c] // 128
                        kbase = koffs[c] // 128
                        ctxc = [ccpool.tile([128, 512], MMDT, tag=f"cc{p}",
                                            name=f"cc{p}") for p in range(2)]
                        state[c] = ctxc
                        dn = btpool.tile([1, 2048], F32, tag="dn", name="dn")
                        rcp = btpool.tile([1, 2048], F32, tag="rcp", name="rcp")
                        for pair in range(2):
                            es = {}      # (hh, ki) -> (tile, slice)
                            # scores for both heads of the pair are
                            # interleaved: the two heads sit on PE row
                            # groups 0-1 / 2-3 (K=64), so adjacent matmuls
                            # execute concurrently on the array.
                            for ki in range(0, nkc, 2):
                                nk2 = min(2, nkc - ki)
                                ps_p = [psB.tile([128, 1024], F32, tag="ps_s",
                                                 name=f"ps_s{i}")
                                        for i in range(2)]
                                e_p = [epool.tile([128, 1024], MMDT, tag="e",
                                                  name=f"e{i}")
                                       for i in range(2)]
                                for kj in range(nk2):
                                    ko = koffs[c] + (ki + kj) * 128
                                    for hh in range(2):
                                        rb = hh * 64
                                        nc.tensor.matmul(
                                            ps_p[hh][:, kj * 512: kj * 512 + nqp],
                                            kT[pair][rb:rb + 64, ko:ko + 128],
                                            qT[pair][rb:rb + 64, qo:qo + nqp],
                                            start=True, stop=True)
                                        es[(hh, ki + kj)] = (
                                            e_p[hh], slice(kj * 512, kj * 512 + nqp))
                                for hh in range(2):
                                    pv = ps_p[hh][:].rearrange(
                                        "p (b n) -> p b n", b=2)[:, 0:nk2, 0:nqp]
                                    ev = e_p[hh][:].rearrange(
                                        "p (b n) -> p b n", b=2)[:, 0:nk2, 0:nqp]
                                    nc.scalar.activation(ev, pv, EXP)
                                pull()
                            pull()
                            for hh in range(2):
                                h = 2 * pair + hh
                                rb = hh * 64
                                ps_c = psC.tile([128, 512], F32, tag="ps_c")
                                for ki in range(nkc):
                                    e, sl = es[(hh, ki)]
                                    nc.tensor.matmul(
                                        ps_c[:65, :nqp],
                                        vA[:, (kbase + ki) * 260 + h * 65:
                                           (kbase + ki) * 260 + (h + 1) * 65],
                                        e[:, sl],
                                        start=(ki == 0), stop=(ki == nkc - 1))
                                # denominator -> ln(denominator) on partition 0
                                nc.scalar.activation(
                                    dn[:, h * 512: h * 512 + nqp],
                                    ps_c[64:65, :nqp], LN)
                                # evacuate UNNORMALIZED right away: frees the
                                # psC bank ~0.6us after the ctx matmul instead
                                # of after the whole ln/rcp/bcast chain, so
                                # the next pair's ctx is never blocked.
                                nc.vector.tensor_copy(
                                    ctxc[pair][rb:rb + 64, :nqp],
                                    ps_c[0:64, :nqp])
                            pull()
                        # reciprocals for all four heads in one ACT call;
                        # bcast+multiply run on GpSimd/Vector while the next
                        # cluster's scores occupy the PE.
                        nc.scalar.activation(
                            rcp[:].rearrange("p (b n) -> p b n", b=4)[:, :, :nqp],
                            dn[:].rearrange("p (b n) -> p b n", b=4)[:, :, :nqp],
                            EXP, scale=-1.0)
                        for h in range(HPC):
                            rb = (h % 2) * 64
                            bt = bbpool.tile([128, 512], F32, tag="bt")
                            nc.gpsimd.partition_broadcast(
                                bt[:, :nqp], rcp[:, h * 512: h * 512 + nqp])
                            nc.vector.tensor_tensor(
                                ctxc[h // 2][rb:rb + 64, :nqp],
                                ctxc[h // 2][rb:rb + 64, :nqp],
                                bt[rb:rb + 64, :nqp], MULT)

                    def oproj_chunk(c, dc, ob):
                        """Transposed projection of one 128-outdim chunk of
                        cluster c: out[od, q] over the whole cluster.  ob is
                        packed cluster-major ([dc*nqp + t]) so the output DMA
                        is fully contiguous (128 descriptors, not 1024)."""
                        qo, nqp = qoffs[c], NQP[c]
                        ctxc = state[c]
                        ps_o = psA.tile([128, 512], F32, tag="psproj")
                        for pair in range(2):
                            nc.tensor.matmul(
                                ps_o[:, :nqp],
                                wo[:, pair * 1024 + dc * 128: pair * 1024 + (dc + 1) * 128],
                                ctxc[pair][:, :nqp],
                                start=(pair == 0), stop=(pair == 1))
                        nc.vector.tensor_copy(ob[:, dc * nqp: (dc + 1) * nqp],
                                              ps_o[:, :nqp])

                    def oproj_emit(c, ob):
                        qo, nqp = qoffs[c], NQP[c]
                        nc.sync.dma_start(OUTT[:, 8 * qo: 8 * qo + 8 * nqp],
                                          ob[:, 0:8 * nqp])

                    cmin = int(np.argmin(NQP))
                    order = [c for c in range(M) if c != cmin] + [cmin]
                    prev = None
                    ob_prev = None
                    for idx, c in enumerate(order):
                        qn = qoffs[c] + NQP[c]
                        kn = min(koffs[c] + NKP[c], kmax)
                        pump_dma(qn, kn)
                        # prefetch the NEXT cluster's block DMAs too so their
                        # compute thunks are available as attention filler
                        if idx + 1 < len(order):
                            c2 = order[idx + 1]
                            pump_dma(qoffs[c2] + NQP[c2],
                                     min(koffs[c2] + NKP[c2], kmax))
                        drain_compute(qn, kn)
                        # filler: previous cluster's oproj chunk-pairs first,
                        # then pending projection thunks of future blocks.
                        odc = [0]

                        def pull(p=prev, ob=ob_prev, odc=odc):
                            if p is not None and odc[0] < 8:
                                oproj_chunk(p, odc[0], ob)
                                oproj_chunk(p, odc[0] + 1, ob)
                                odc[0] += 2
                            else:
                                pull_any()
                        attn(c, pull)
                        if prev is not None:
                            while odc[0] < 8:
                                oproj_chunk(prev, odc[0], ob_prev)
                                odc[0] += 1
                            oproj_emit(prev, ob_prev)
                            state.pop(prev)
                        prev = c
                        ob_prev = opool.tile([128, 8 * 512], F16, tag="ob",
                                             name="ob")
                    qo_l, nqp_l = qoffs[prev], NQP[prev]
                    for dc in range(8):
                        oproj_chunk(prev, dc, ob_prev)
                        if dc % 2 == 1:
                            # eager quarter-emits shorten the output drain
                            nc.sync.dma_start(
                                OUTT[:, 8 * qo_l + (dc - 1) * nqp_l:
                                     8 * qo_l + (dc + 1) * nqp_l],
                                ob_prev[:, (dc - 1) * nqp_l:(dc + 1) * nqp_l])
                    state.pop(prev)

    nc.compile()
    return nc


_CACHE = {}
_WARM = {}


def run(inputs, trace=False):
    queries = np.asarray(inputs["queries"], np.float32)
    keys = np.asarray(inputs["keys"], np.float32)
    values = np.asarray(inputs["values"], np.float32)
    Wq = np.asarray(inputs["Wq"], np.float32)
    Wk = np.asarray(inputs["Wk"], np.float32)
    Wv = np.asarray(inputs["Wv"], np.float32)
    Wo = np.asarray(inputs["Wo"], np.float32)
    Wr = np.asarray(inputs["Wr"], np.float32)

    B, LQ, D = queries.shape
    M = Wr.shape[1]
    DH = D // H
    scale = np.float32(1.0 / np.sqrt(DH))

    aq = np.argmax(queries @ Wr, axis=-1)   # [B, LQ]
    ak = np.argmax(keys @ Wr, axis=-1)      # [B, LK]

    NQP, NKP, qoffs, koffs, LQG, NKG = _plan(aq, ak, M)
    NVC = NKG // 128
    kreal_max = [int(max((ak[b] == c).sum() for b in range(B))) for c in range(M)]

    key = (tuple(NQP), tuple(NKP), tuple(kreal_max), LQG, NKG, D, str(MMDT))
    if key not in _CACHE:
        _CACHE[key] = _build_program(NQP, NKP, qoffs, koffs, LQG, NKG, D, kreal_max)
    nc = _CACHE[key]

    # ---- gather + pad, build per-batch inputs ----
    perm_q = []   # original token ids, per batch, in gathered order
    slot_q = []   # gathered positions of those tokens
    XQTs, XKTs, XVTs, ONEs = [], [], [], []
    for b in range(B):
        xq = np.zeros((LQG, D), np.float32)
        xk = np.zeros((NKG, D), np.float32)
        xv = np.zeros((NKG, D), np.float32)
        kreal = np.zeros(NKG, np.float32)
        pq, sq = [], []
        for c in range(M):
            tq = np.nonzero(aq[b] == c)[0]
            tk = np.nonzero(ak[b] == c)[0]
            xq[qoffs[c]:qoffs[c] + len(tq)] = queries[b, tq]
            xk[koffs[c]:koffs[c] + len(tk)] = keys[b, tk]
            xv[koffs[c]:koffs[c] + len(tk)] = values[b, tk]
            kreal[koffs[c]:koffs[c] + len(tk)] = 1.0
            pq.append(tq)
            sq.append(np.arange(qoffs[c], qoffs[c] + len(tq)))
        perm_q.append(np.concatenate(pq))
        slot_q.append(np.concatenate(sq))
        XQTs.append(pack_x(xq))
        XKTs.append(pack_x(xk))
        XVTs.append(pack_x(xv))
        # indicator per chunk/partition, replicated across the 4 heads
        ind = kreal.reshape(NVC, 128).T  # [128, NVC]
        ONEs.append(np.ascontiguousarray(
            np.repeat(ind[:, :, None], HPC, axis=2).reshape(128, NVC * HPC)
        ).astype(NPDT))

    in_maps = []
    for core in range(N_CORES):
        b, hg = core // HPC, core % HPC
        cols = slice(hg * HPC * DH, (hg + 1) * HPC * DH)
        in_maps.append({
            "XQT": XQTs[b], "XKT": XKTs[b], "XVT": XVTs[b],
            "WQ": pack_w(Wq[:, cols] * scale),
            "WK": pack_w(Wk[:, cols]),
            "WV": pack_w(Wv[:, cols]),
            "WO": pack_wo(Wo[cols, :]),
            "ONE": ONEs[b],
        })

    # Warmup execution: the very first execution of a freshly loaded NEFF
    # can race its own input staging (cold axon/PJRT path) and corrupt
    # results; every execution after the first is clean (verified with
    # CoreSim: the program itself is race-free).  Run once, discard, and
    # return the second execution's results.
    global _WARM
    if not _WARM.get(id(nc)):
        run_bass_kernel_spmd(nc, in_maps, list(range(N_CORES)), trace=False)
        _WARM[id(nc)] = True
    res = run_bass_kernel_spmd(nc, in_maps, list(range(N_CORES)), trace=trace)

    out = np.zeros((B, LQ, D), np.float32)
    for b in range(B):
        acc = res.results[b * HPC]["OUTT"].astype(np.float32)
        for hg in range(1, HPC):
            acc += res.results[b * HPC + hg]["OUTT"].astype(np.float32)
        # cluster-major: OUTT[p, 8*qoffs[c] + dc*nqp + t] = out[qoffs[c]+t, dc*128+p]
        full = np.zeros((LQG, D), np.float32)
        for c in range(M):
            qo, nqp = qoffs[c], NQP[c]
            blk = acc[:, 8 * qo: 8 * qo + 8 * nqp].reshape(128, 8, nqp)
            full[qo:qo + nqp] = blk.transpose(2, 1, 0).reshape(nqp, D)
        out[b, perm_q[b]] = full[slot_q[b]]
    return out, res


def kernel(**inputs):
    out, _ = run(inputs)
    return out


# revision 34
# speedup vs baseline: 1.0885x; 1.0885x over previous
"""MoE clustered attention kernel for Trainium2 (8 NeuronCores).

Problem: B=2, LQ=LK=2048, D=1024, H=16 heads (DH=64), M=8 clusters.
Each query/key token is routed (argmax of X @ Wr) to one of 8 clusters;
attention is only computed within a cluster (block-sparse attention).

Strategy (v4)
-------------
Host side:
  * compute router assignments with numpy fp32,
  * gather tokens by cluster into contiguous ranges (queries rounded to
    4, keys to multiples of 128) so one SPMD program serves both
    batches; everything is fp16,
  * pre-transpose X to [D, L] block-major; pre-pack weights into the
    on-chip [128, 2048] layout so weight DMAs are fully contiguous,
  * a per-key-chunk "real key" indicator column rides in vA at slot 64:
    padded keys contribute exp(0)*0 to the denominator and 0*v to the
    numerator, so no mask rows are needed at all (K=64 contraction).

Device side (per core; core = batch * 4 + head_group, 4 heads each):
  * weight DMAs issue on the Scalar hardware-DGE queue, X blocks on the
    Sync queue, so the first projection matmul starts ~10us earlier,
  * per-PAIR qT/kT projections in transposed layout [128, L] (head 2p
    on partitions 0:64, head 2p+1 on 64:128) with N<=512 moving
    matmuls; ONE full-partition evacuation per pair-block on VectorE;
    v in natural layout with the indicator column making the ctx
    matmul emit the softmax denominator as row 64,
  * attention iterates cluster-outer / pair-inner: scores for both
    heads of the pair, batched exp on ScalarE (ScalarE runs ONLY exp
    during attention - no FIFO head-of-line blocking), ctx matmul,
    then the denominator row is staged to partition 0 (VectorE),
    reciprocal'd (VectorE), broadcast (GpSimdE) and multiplied into
    the ctx evacuation in a single fused VectorE op,
  * output projection is transposed (stationary = Wo 128-dim chunk,
    moving = the cluster's nqp tokens): 16 matmuls of N=nqp per
    cluster, evacuated into a [128, 8*512] staging tile and DMA'd out
    in one descriptor per cluster; chunks of the previous cluster are
    interleaved into the exp-latency slots of the current cluster.
Host sums the 4 head-group partials per batch in fp32 and un-permutes.
"""

import numpy as np

import concourse.bacc as bacc
import concourse.tile as tile
import concourse.mybir as mybir
import concourse.hw_specs as hw_specs
from concourse.bass_utils import run_bass_kernel_spmd

F32 = mybir.dt.float32
F16 = mybir.dt.float16
EXP = mybir.ActivationFunctionType.Exp
LN = mybir.ActivationFunctionType.Ln
MULT = mybir.AluOpType.mult

H = 16            # total heads
HPC = 4           # heads per core
N_CORES = 8

MMDT = F16
NPDT = np.float16

# Route Exp and Ln to the one ACT table set that contains both, so the
# table-load insertion pass emits a single load instead of thrashing
# between exp_and_others and natural_log on every call.
_orig_get_activation_tables = hw_specs.get_activation_tables


def _patched_get_activation_tables(arch):
    out = {}
    for name, fns in _orig_get_activation_tables(arch).items():
        fns = set(fns)
        if name != "natural_log_exp_and_others":
            fns.discard(EXP)
            fns.discard(LN)
        out[name] = fns
    return out


bacc.get_activation_tables = _patched_get_activation_tables


def _ceil_to(x, m):
    return (x + m - 1) // m * m


def pack_x(x):
    # [L, D] -> block-major [128, NB, 8, 512] so one 512-token block is
    # an 8KB-contiguous slab per partition
    L = x.shape[0]
    nb = _ceil_to(L, 512) // 512
    xp = np.zeros((128, 8, nb * 512), np.float32)
    xp[:, :, :L] = x.T.reshape(8, 128, L).transpose(1, 0, 2)
    return np.ascontiguousarray(
        xp.reshape(128, 8, nb, 512).transpose(0, 2, 1, 3).reshape(128, -1)
    ).astype(NPDT)


def pack_w(w):
    # [1024, 256] -> on-chip [128, 8*256] with w[p, d*256+m] = W[d*128+p, m]
    return np.ascontiguousarray(
        w.reshape(8, 128, 256).transpose(1, 0, 2).reshape(128, 2048)).astype(NPDT)


def pack_wo(w):
    # [256, 1024] -> [128, 2*1024] with wo[p, n*1024+m] = W[n*128+p, m]
    return np.ascontiguousarray(
        w.reshape(2, 128, 1024).transpose(1, 0, 2).reshape(128, 2048)).astype(NPDT)


def _plan(aq, ak, M):
    """Common (cross-batch) padded cluster geometry."""
    B = aq.shape[0]
    nq = np.array([[int((aq[b] == c).sum()) for c in range(M)] for b in range(B)])
    nk = np.array([[int((ak[b] == c).sum()) for c in range(M)] for b in range(B)])
    NQP = [max(64, _ceil_to(int(nq[:, c].max()), 4)) for c in range(M)]
    NKP = [_ceil_to(max(128, int(nk[:, c].max())), 128) for c in range(M)]
    qoff = np.concatenate([[0], np.cumsum(NQP)])
    koff = np.concatenate([[0], np.cumsum(NKP)])
    LQG = _ceil_to(int(qoff[-1]), 4)
    NKG = _ceil_to(int(koff[-1]), 128)
    return NQP, NKP, qoff[:-1].tolist(), koff[:-1].tolist(), LQG, NKG


def _build_program(NQP, NKP, qoffs, koffs, LQG, NKG, D, kreal_max):
    nc = bacc.Bacc("TRN2", target_bir_lowering=False, debug=False)
    ND = D // 128
    NBQ = _ceil_to(LQG, 512) // 512
    NBK = _ceil_to(NKG, 512) // 512
    # block-major layout [128, nblocks, 8, 512]: one 512-token block is an
    # 8KB-contiguous slab per partition -> 128 DMA descriptors instead of 1024
    XQT = nc.dram_tensor("XQT", [128, NBQ * ND * 512], MMDT, kind="ExternalInput").ap()
    XKT = nc.dram_tensor("XKT", [128, NBK * ND * 512], MMDT, kind="ExternalInput").ap()
    XVT = nc.dram_tensor("XVT", [128, NBK * ND * 512], MMDT, kind="ExternalInput").ap()
    WQ = nc.dram_tensor("WQ", [128, 2048], MMDT, kind="ExternalInput").ap()
    WK = nc.dram_tensor("WK", [128, 2048], MMDT, kind="ExternalInput").ap()
    WV = nc.dram_tensor("WV", [128, 2048], MMDT, kind="ExternalInput").ap()
    WO = nc.dram_tensor("WO", [128, 2048], MMDT, kind="ExternalInput").ap()
    NVC = NKG // 128       # value token chunks
    ONE = nc.dram_tensor("ONE", [128, NVC * HPC], MMDT, kind="ExternalInput").ap()
    # transposed output: OUTT[p, dc*LQG + t] = out[t, dc*128 + p]
    OUTT = nc.dram_tensor("OUTT", [128, 8 * LQG], F16, kind="ExternalOutput").ap()

    M = len(NQP)

    def xview(xdram, nb):
        return xdram.rearrange("p (b n m) -> p b n m", b=nb, n=ND)

    with tile.TileContext(nc) as tc:
        with (
            tc.tile_pool(name="weights", bufs=1) as wpool,
            tc.tile_pool(name="proj_out", bufs=1) as projpool,
            tc.tile_pool(name="psA", bufs=2, space="PSUM") as psA,
            tc.tile_pool(name="psB", bufs=2, space="PSUM") as psB,
            tc.tile_pool(name="psC", bufs=2, space="PSUM") as psC,
        ):
            wq = wpool.tile([128, 2048], MMDT, tag="wq")
            wk = wpool.tile([128, 2048], MMDT, tag="wk")
            wv = wpool.tile([128, 2048], MMDT, tag="wv")
            wo = wpool.tile([128, 2048], MMDT, tag="wo")

            # pair-layout projections: head 2p on partitions 0:64,
            # head 2p+1 on partitions 64:128
            qT = [projpool.tile([128, LQG], MMDT, tag=f"qT{p}", name=f"qT{p}")
                  for p in range(2)]
            kT = [projpool.tile([128, NKG], MMDT, tag=f"kT{p}", name=f"kT{p}")
                  for p in range(2)]
            vA = projpool.tile([128, NVC * 260], MMDT, tag="vA")

            # wq/wk/ones ride the Scalar hwdge queue so the Sync queue's
            # first issues are the first X sub-blocks.  Later weights are
            # issued from the Sync queue BETWEEN x-blocks: the DMA engines
            # round-robin everything enqueued, so issue order is the only
            # control over which transfers get bandwidth first.
            nc.scalar.dma_start(wq[:, 0:1024], WQ[:, 0:1024])
            nc.scalar.dma_start(wq[:, 1024:2048], WQ[:, 1024:2048])
            nc.scalar.dma_start(wk[:], WK)

            vA_c = vA[:].rearrange("p (c h e) -> p c h e", c=NVC, h=HPC)
            # real-key indicator column (vA slot 64 per chunk/head): DMA
            # contiguously into a staging tile (a strided 2-byte-element DMA
            # costs ~33us in descriptor processing!), then one fast DVE
            # strided copy into vA.
            ones_sb = projpool.tile([128, NVC * HPC], MMDT, tag="ones_sb")
            nc.scalar.dma_start(ones_sb[:], ONE)
            nc.vector.tensor_copy(vA_c[:, :, :, 64:65], ones_sb[:])

            # ---- projections ----
            # Block DMA and block compute are DECOUPLED: pump_dma() issues a
            # block's DMA (+ queues its compute thunks) as early as possible;
            # thunks are emitted either in bulk (drain, before the cluster
            # that needs them) or one at a time inside the attention's
            # exp-latency slots, so TensorE never idles long enough for the
            # HAM clock gate to re-throttle.
            with tc.tile_pool(name="xin", bufs=8) as xpool:
                def q_pair(xt, off, pair):
                    w = min(512, LQG - off)
                    ps = psA.tile([128, 512], F32, tag="psproj")
                    for d in range(ND):
                        nc.tensor.matmul(
                            ps[:, :w],
                            wq[:, d * 256 + pair * 128: d * 256 + (pair + 1) * 128],
                            xt[:, d, :w],
                            start=(d == 0), stop=(d == ND - 1))
                    nc.vector.tensor_copy(qT[pair][:, off:off + w], ps[:, :w])

                def k_pair(xt, off, pair):
                    w = min(512, NKG - off)
                    ps = psA.tile([128, 512], F32, tag="psproj")
                    for d in range(ND):
                        nc.tensor.matmul(
                            ps[:, :w],
                            wk[:, d * 256 + pair * 128: d * 256 + (pair + 1) * 128],
                            xt[:, d, :w],
                            start=(d == 0), stop=(d == ND - 1))
                    nc.vector.tensor_copy(kT[pair][:, off:off + w], ps[:, :w])

                def v_sub(xt, off, sub2):
                    w = min(512, NKG - off)
                    n2 = min(2, w // 128 - sub2)
                    tc128 = off // 128 + sub2
                    ps = psA.tile([128, 512], F32, tag="psproj")
                    for s in range(n2):
                        for d in range(ND):
                            nc.tensor.matmul(ps[:, s * 256:(s + 1) * 256],
                                             xt[:, d, (sub2 + s) * 128:(sub2 + s + 1) * 128],
                                             wv[:, d * 256:(d + 1) * 256],
                                             start=(d == 0), stop=(d == ND - 1))
                    nc.vector.tensor_copy(
                        vA_c[:, tc128:tc128 + n2, :, 0:64],
                        ps[:, :n2 * 256].rearrange("p (c h e) -> p c h e", c=n2, h=HPC))

                # static padding segments of the gathered key axis: tokens
                # [koffs[c]+kreal_max[c], koffs[c]+NKP[c]) are zero for BOTH
                # batches - skip their DMA entirely and zero them on-chip.
                pad_ranges = []
                for c in range(M):
                    a = koffs[c] + kreal_max[c]
                    bb = koffs[c] + NKP[c]
                    if a < bb:
                        pad_ranges.append((a, bb))

                def kv_dma(xt, xdram, off, w):
                    xv = xview(xdram, NBK)
                    pos = off
                    for (a, bb) in pad_ranges:
                        a2, b2 = max(a, off), min(bb, off + w)
                        if a2 >= b2:
                            continue
                        if pos < a2:
                            nc.sync.dma_start(
                                xt[:, :, pos - off:a2 - off],
                                xv[:, off // 512, :, pos - off:a2 - off])
                        nc.gpsimd.memset(xt[:, :, a2 - off:b2 - off], 0.0)
                        pos = b2
                    if pos < off + w:
                        nc.sync.dma_start(
                            xt[:, :, pos - off:w],
                            xv[:, off // 512, :, pos - off:w])

                # interleaved block DMA schedule: q/k/v alternate so the DMA
                # engines always work ~1 block ahead of the PE; weights are
                # enqueued between blocks (the DMA engines round-robin all
                # enqueued transfers, so issue order controls bandwidth).
                sched = []
                nqb = (LQG + 511) // 512
                nkb = (NKG + 511) // 512
                for i in range(max(nqb, nkb)):
                    if i < nqb:
                        sched.append(("q", i * 512))
                    if i < nkb:
                        if i == 0:
                            sched.append(("wk", 0))
                        sched.append(("k", i * 512))
                        if i == 0:
                            sched.append(("wv", 0))
                        if i == 1:
                            sched.append(("wo", 0))
                        sched.append(("v", i * 512))
                dma_done = {"q": 0, "k": 0, "v": 0}
                comp_done = {"q": 0, "k": 0, "v": 0}
                pending = {"q": [], "k": [], "v": []}
                sched_pos = [0]

                def issue_next():
                    kind, off = sched[sched_pos[0]]
                    sched_pos[0] += 1
                    if kind == "wk":
                        nc.sync.dma_start(wk[:], WK)
                        return
                    if kind == "wv":
                        nc.sync.dma_start(wv[:], WV)
                        return
                    if kind == "wo":
                        nc.sync.dma_start(wo[:], WO)
                        return
                    xt = xpool.tile([128, ND, 512], MMDT, tag="xt", name="xt")
                    if kind == "q":
                        xv = xview(XQT, NBQ)
                        if off == 0:
                            # split the first block so the first matmul only
                            # waits for a quarter of it
                            for sd in range(0, ND, 2):
                                nc.sync.dma_start(xt[:, sd:sd + 2, :],
                                                  xv[:, 0, sd:sd + 2, :])
                        else:
                            nc.sync.dma_start(xt[:, :, :], xv[:, off // 512, :, :])
                        thunks = [lambda p=p, x=xt, o=off: q_pair(x, o, p)
                                  for p in range(2)]
                    elif kind == "k":
                        kv_dma(xt, XKT, off, min(512, NKG - off))
                        thunks = [lambda p=p, x=xt, o=off: k_pair(x, o, p)
                                  for p in range(2)]
                    else:
                        w = min(512, NKG - off)
                        kv_dma(xt, XVT, off, w)
                        thunks = [lambda s=s, x=xt, o=off: v_sub(x, o, s)
                                  for s in range(0, w // 128, 2)]
                    pending[kind].append((off, thunks))
                    dma_done[kind] = off + 512

                def pump_dma(qneed, kvneed):
                    while (dma_done["q"] < qneed or dma_done["k"] < kvneed
                           or dma_done["v"] < kvneed) and sched_pos[0] < len(sched):
                        issue_next()

                def drain_one(kind):
                    off, thunks = pending[kind][0]
                    thunks.pop(0)()
                    if not thunks:
                        pending[kind].pop(0)
                        comp_done[kind] = off + 512

                def drain_compute(qneed, kvneed):
                    for kind, need in (("q", qneed), ("k", kvneed), ("v", kvneed)):
                        while comp_done[kind] < need and pending[kind]:
                            drain_one(kind)

                def pull_any():
                    for kind in ("k", "v", "q"):
                        if pending[kind]:
                            drain_one(kind)
                            return

                # ---- cluster-ordered K/V projection + attention + output ----
                # K/V blocks are emitted just before the first cluster that
                # needs them, so TensorE runs one dense stream through the
                # whole kernel (keeps the HAM clock warm); ctx lives in small
                # per-cluster tiles so cluster c's output projection never
                # write-after-read blocks cluster c+1's ctx.
                with tc.tile_pool(name="epool", bufs=6) as epool, \
                     tc.tile_pool(name="ccpool", bufs=3) as ccpool, \
                     tc.tile_pool(name="btpool", bufs=3) as btpool, \
                     tc.tile_pool(name="bbpool", bufs=4) as bbpool, \
                     tc.tile_pool(name="outsb", bufs=2) as opool:
                    kmax = koffs[M - 1] + NKP[M - 1]
                    state = {}

                    def attn(c, pull):
                        """Scores/exp/ctx/normalize for the 4 heads of
                        cluster c, pair at a time.  `pull()` emits one unit
                        of ready TensorE work (prev cluster's oproj or a
                        pending projection thunk) into each exp/denominator
                        latency slot."""
                        qo, nqp = qoffs[c], NQP[c]
                        nkc = NKP[c] // 128
                        kbase = koffs[c] // 128
                        ctxc = [ccpool.tile([128, 512], MMDT, tag=f"cc{p}",
                                            name=f"cc{p}") for p in range(2)]
                        state[c] = ctxc
                        for pair in range(2):
                            es = {}      # (hh, ki) -> (tile, slice)
                            # scores for both heads of the pair are
                            # interleaved: the two heads sit on PE row
                            # groups 0-1 / 2-3 (K=64), so adjacent matmuls
                            # execute concurrently on the array.
                            for ki in range(0, nkc, 2):
                                nk2 = min(2, nkc - ki)
                                ps_p = [psB.tile([128, 1024], F32, tag="ps_s",
                                                 name=f"ps_s{i}")
                                        for i in range(2)]
                                e_p = [epool.tile([128, 1024], MMDT, tag="e",
                                                  name=f"e{i}")
                                       for i in range(2)]
                                for kj in range(nk2):
                                    ko = koffs[c] + (ki + kj) * 128
                                    for hh in range(2):
                                        rb = hh * 64
                                        nc.tensor.matmul(
                                            ps_p[hh][:, kj * 512: kj * 512 + nqp],
                                            kT[pair][rb:rb + 64, ko:ko + 128],
                                            qT[pair][rb:rb + 64, qo:qo + nqp],
                                            start=True, stop=True)
                                        es[(hh, ki + kj)] = (
                                            e_p[hh], slice(kj * 512, kj * 512 + nqp))
                                for hh in range(2):
                                    pv = ps_p[hh][:].rearrange(
                                        "p (b n) -> p b n", b=2)[:, 0:nk2, 0:nqp]
                                    ev = e_p[hh][:].rearrange(
                                        "p (b n) -> p b n", b=2)[:, 0:nk2, 0:nqp]
                                    nc.scalar.activation(ev, pv, EXP)
                                pull()
                            pull()
                            dn = btpool.tile([1, 1024], F32, tag="dn", name="dn")
                            rcp = btpool.tile([1, 1024], F32, tag="rcp", name="rcp")
                            for hh in range(2):
                                h = 2 * pair + hh
                                rb = hh * 64
                                ps_c = psC.tile([128, 512], F32, tag="ps_c")
                                for ki in range(nkc):
                                    e, sl = es[(hh, ki)]
                                    nc.tensor.matmul(
                                        ps_c[:65, :nqp],
                                        vA[:, (kbase + ki) * 260 + h * 65:
                                           (kbase + ki) * 260 + (h + 1) * 65],
                                        e[:, sl],
                                        start=(ki == 0), stop=(ki == nkc - 1))
                                # denominator -> ln(denominator) on partition 0
                                nc.scalar.activation(
                                    dn[:, hh * 512: hh * 512 + nqp],
                                    ps_c[64:65, :nqp], LN)
                                # evacuate UNNORMALIZED right away: frees the
                                # psC bank ~0.6us after the ctx matmul instead
                                # of after the whole ln/rcp/bcast chain, so
                                # the next pair's ctx is never blocked.
                                nc.vector.tensor_copy(
                                    ctxc[pair][rb:rb + 64, :nqp],
                                    ps_c[0:64, :nqp])
                            pull()
                            # reciprocals for both heads in one ACT call
                            nc.scalar.activation(
                                rcp[:].rearrange("p (b n) -> p b n", b=2)[:, :, :nqp],
                                dn[:].rearrange("p (b n) -> p b n", b=2)[:, :, :nqp],
                                EXP, scale=-1.0)
                            for hh in range(2):
                                rb = hh * 64
                                bt = bbpool.tile([128, 512], F32, tag="bt")
                                nc.gpsimd.partition_broadcast(
                                    bt[:, :nqp], rcp[:, hh * 512: hh * 512 + nqp])
                                nc.vector.tensor_tensor(
                                    ctxc[pair][rb:rb + 64, :nqp],
                                    ctxc[pair][rb:rb + 64, :nqp],
                                    bt[rb:rb + 64, :nqp], MULT)

                    def oproj_chunk(c, dc, ob):
                        """Transposed projection of one 128-outdim chunk of
                        cluster c: out[od, q] over the whole cluster.  ob is
                        packed cluster-major ([dc*nqp + t]) so the output DMA
                        is fully contiguous (128 descriptors, not 1024)."""
                        qo, nqp = qoffs[c], NQP[c]
                        ctxc = state[c]
                        ps_o = psA.tile([128, 512], F32, tag="psproj")
                        for pair in range(2):
                            nc.tensor.matmul(
                                ps_o[:, :nqp],
                                wo[:, pair * 1024 + dc * 128: pair * 1024 + (dc + 1) * 128],
                                ctxc[pair][:, :nqp],
                                start=(pair == 0), stop=(pair == 1))
                        nc.vector.tensor_copy(ob[:, dc * nqp: (dc + 1) * nqp],
                                              ps_o[:, :nqp])

                    def oproj_emit(c, ob):
                        qo, nqp = qoffs[c], NQP[c]
                        nc.sync.dma_start(OUTT[:, 8 * qo: 8 * qo + 8 * nqp],
                                          ob[:, 0:8 * nqp])

                    cmin = int(np.argmin(NQP))
                    order = [c for c in range(M) if c != cmin] + [cmin]
                    prev = None
                    ob_prev = None
                    for idx, c in enumerate(order):
                        qn = qoffs[c] + NQP[c]
                        kn = min(koffs[c] + NKP[c], kmax)
                        pump_dma(qn, kn)
                        # prefetch the NEXT cluster's block DMAs too so their
                        # compute thunks are available as attention filler
                        if idx + 1 < len(order):
                            c2 = order[idx + 1]
                            pump_dma(qoffs[c2] + NQP[c2],
                                     min(koffs[c2] + NKP[c2], kmax))
                        drain_compute(qn, kn)
                        # filler: previous cluster's oproj chunk-pairs first,
                        # then pending projection thunks of future blocks.
                        odc = [0]

                        def pull(p=prev, ob=ob_prev, odc=odc):
                            if p is not None and odc[0] < 8:
                                oproj_chunk(p, odc[0], ob)
                                oproj_chunk(p, odc[0] + 1, ob)
                                odc[0] += 2
                            else:
                                pull_any()
                        attn(c, pull)
                        if prev is not None:
                            while odc[0] < 8:
                                oproj_chunk(prev, odc[0], ob_prev)
                                odc[0] += 1
                            oproj_emit(prev, ob_prev)
                            state.pop(prev)
                        prev = c
                        ob_prev = opool.tile([128, 8 * 512], F16, tag="ob",
                                             name="ob")
                    qo_l, nqp_l = qoffs[prev], NQP[prev]
                    for dc in range(8):
                        oproj_chunk(prev, dc, ob_prev)
                        if dc == 3:
                            # early half-emit shortens the output-drain tail
                            nc.sync.dma_start(
                                OUTT[:, 8 * qo_l: 8 * qo_l + 4 * nqp_l],
                                ob_prev[:, 0:4 * nqp_l])
                    nc.sync.dma_start(
                        OUTT[:, 8 * qo_l + 4 * nqp_l: 8 * qo_l + 8 * nqp_l],
                        ob_prev[:, 4 * nqp_l:8 * nqp_l])
                    state.pop(prev)

    nc.compile()
    return nc


_CACHE = {}
_WARM = {}


def run(inputs, trace=False):
    queries = np.asarray(inputs["queries"], np.float32)
    keys = np.asarray(inputs["keys"], np.float32)
    values = np.asarray(inputs["values"], np.float32)
    Wq = np.asarray(inputs["Wq"], np.float32)
    Wk = np.asarray(inputs["Wk"], np.float32)
    Wv = np.asarray(inputs["Wv"], np.float32)
    Wo = np.asarray(inputs["Wo"], np.float32)
    Wr = np.asarray(inputs["Wr"], np.float32)

    B, LQ, D = queries.shape
    M = Wr.shape[1]
    DH = D // H
    scale = np.float32(1.0 / np.sqrt(DH))

    aq = np.argmax(queries @ Wr, axis=-1)   # [B, LQ]
    ak = np.argmax(keys @ Wr, axis=-1)      # [B, LK]

    NQP, NKP, qoffs, koffs, LQG, NKG = _plan(aq, ak, M)
    NVC = NKG // 128
    kreal_max = [int(max((ak[b] == c).sum() for b in range(B))) for c in range(M)]

    key = (tuple(NQP), tuple(NKP), tuple(kreal_max), LQG, NKG, D, str(MMDT))
    if key not in _CACHE:
        _CACHE[key] = _build_program(NQP, NKP, qoffs, koffs, LQG, NKG, D, kreal_max)
    nc = _CACHE[key]

    # ---- gather + pad, build per-batch inputs ----
    perm_q = []   # original token ids, per batch, in gathered order
    slot_q = []   # gathered positions of those tokens
    XQTs, XKTs, XVTs, ONEs = [], [], [], []
    for b in range(B):
        xq = np.zeros((LQG, D), np.float32)
        xk = np.zeros((NKG, D), np.float32)
        xv = np.zeros((NKG, D), np.float32)
        kreal = np.zeros(NKG, np.float32)
        pq, sq = [], []
        for c in range(M):
            tq = np.nonzero(aq[b] == c)[0]
            tk = np.nonzero(ak[b] == c)[0]
            xq[qoffs[c]:qoffs[c] + len(tq)] = queries[b, tq]
            xk[koffs[c]:koffs[c] + len(tk)] = keys[b, tk]
            xv[koffs[c]:koffs[c] + len(tk)] = values[b, tk]
            kreal[koffs[c]:koffs[c] + len(tk)] = 1.0
            pq.append(tq)
            sq.append(np.arange(qoffs[c], qoffs[c] + len(tq)))
        perm_q.append(np.concatenate(pq))
        slot_q.append(np.concatenate(sq))
        XQTs.append(pack_x(xq))
        XKTs.append(pack_x(xk))
        XVTs.append(pack_x(xv))
        # indicator per chunk/partition, replicated across the 4 heads
        ind = kreal.reshape(NVC, 128).T  # [128, NVC]
        ONEs.append(np.ascontiguousarray(
            np.repeat(ind[:, :, None], HPC, axis=2).reshape(128, NVC * HPC)
        ).astype(NPDT))

    in_maps = []
    for core in range(N_CORES):
        b, hg = core // HPC, core % HPC
        cols = slice(hg * HPC * DH, (hg + 1) * HPC * DH)
        in_maps.append({
            "XQT": XQTs[b], "XKT": XKTs[b], "XVT": XVTs[b],
            "WQ": pack_w(Wq[:, cols] * scale),
            "WK": pack_w(Wk[:, cols]),
            "WV": pack_w(Wv[:, cols]),
            "WO": pack_wo(Wo[cols, :]),
            "ONE": ONEs[b],
        })

    # Warmup execution: the very first execution of a freshly loaded NEFF
    # can race its own input staging (cold axon/PJRT path) and corrupt
    # results; every execution after the first is clean (verified with
    # CoreSim: the program itself is race-free).  Run once, discard, and
    # return the second execution's results.
    global _WARM
    if not _WARM.get(id(nc)):
        run_bass_kernel_spmd(nc, in_maps, list(range(N_CORES)), trace=False)
        _WARM[id(nc)] = True
    res = run_bass_kernel_spmd(nc, in_maps, list(range(N_CORES)), trace=trace)

    out = np.zeros((B, LQ, D), np.float32)
    for b in range(B):
        acc = res.results[b * HPC]["OUTT"].astype(np.float32)
        for hg in range(1, HPC):
            acc += res.results[b * HPC + hg]["OUTT"].astype(np.float32)
        # cluster-major: OUTT[p, 8*qoffs[c] + dc*nqp + t] = out[qoffs[c]+t, dc*128+p]
        full = np.zeros((LQG, D), np.float32)
        for c in range(M):
            qo, nqp = qoffs[c], NQP[c]
            blk = acc[:, 8 * qo: 8 * qo + 8 * nqp].reshape(128, 8, nqp)
            full[qo:qo + nqp] = blk.transpose(2, 1, 0).reshape(nqp, D)
        out[b, perm_q[b]] = full[slot_q[b]]
    return out, res


def kernel(**inputs):
    out, _ = run(inputs)
    return out
